# revision 2
# baseline (speedup 1.0000x reference)
# Trainium2 Bass kernel for nn_DiffNet.
#
# Math: the conv2(conv1(.)) meta-MLP is affine per element, so with
#   coef = (conv2_w @ conv1_w)[0]  (c0, c1, c2),
#   bc   = (conv2_w @ conv1_b)[0] + conv2_b[0],
#   scale = RATE / batch_num,
# each layer (W, b) of the reference reduces to
#   z  = vi @ W.T                      (pre-bias matmul)
#   vj = relu(z + b)
#   s  = rowsum(vi),  q = rowsum(vi^2)
#   out = (1 + C2*s) * vj + C1*z + (C0*q + Cb*s)
# with C* = scale * (c*, bc).  No [B, out, in] tensor is ever materialized.
#
# Sharding: data-parallel over batch (64 rows -> 8 rows/core), weights
# replicated per core, zero collectives.
#
# Device-side bias folding: PSUM holds P = vi' @ W.T + bhat, where inputs are
# represented as vi = vi' + m (m a constant row vector, m1 = 0) and
# bhat = b + m @ W.T, so P = z + b exactly.  Then
#   out' = alpha (.) relu(P) + C1*P + delta,   out = out' - C1*b,
# so the next layer's constant offset is m_next = -C1*b, folded on host into
# bhat_next, k_alpha, k_delta, and the q cross-term.

import numpy as np

RATE = 0.01
B, IN, H1, H2, OUT = 64, 1024, 512, 512, 256
NCORES = 8
BL = B // NCORES  # 8 rows per core
P128 = 128

# const-vector columns
C_C0, C_C1, C_C2, C_CB, C_2C0 = 0, 1, 2, 3, 4
C_KA0 = 5   # 5,6,7 = k_alpha per layer
C_KD0 = 8   # 8,9,10 = k_delta per layer
NCONST = 11

_NC_CACHE = {}


def _build_nc():
    import concourse.bacc as bacc
    import concourse.mybir as mybir
    import concourse.tile as tile

    fp32 = mybir.dt.float32
    AF = mybir.ActivationFunctionType
    ALU = mybir.AluOpType
    AX = mybir.AxisListType

    nc = bacc.Bacc("TRN2", target_bir_lowering=False, debug=False)

    def din(name, shape):
        return nc.dram_tensor(name, list(shape), fp32, kind="ExternalInput")

    x_t = din("xb", (BL, IN))
    xt_t = din("xtb", (P128, (IN // P128) * BL))
    w_t = [
        din("w1t", (P128, (IN // P128) * H1)),
        din("w2t", (P128, (H1 // P128) * H2)),
        din("w3t", (P128, (H2 // P128) * OUT)),
    ]
    b_t = [din("bh1", (1, H1)), din("bh2", (1, H2)), din("bh3", (1, OUT))]
    m_t = [None, din("m2r", (BL, H1)), din("m3r", (BL, H2))]
    m4_t = din("m4r", (BL, OUT))
    cst_t = din("cst", (BL, NCONST))
    one_t = din("ones1", (1, BL))
    id_t = din("id8", (BL, BL))
    out_t = nc.dram_tensor("outb", [BL, OUT], fp32, kind="ExternalOutput")

    NKS = [IN // P128, H1 // P128, H2 // P128]   # contraction chunks / layer
    NINS = [IN, H1, H2]
    NOUTS = [H1, H2, OUT]

    with tile.TileContext(nc) as tc:
        with (
            tc.tile_pool(name="wp", bufs=1) as wp,
            tc.tile_pool(name="actp", bufs=1) as ap_,
            tc.tile_pool(name="scp", bufs=1) as scp,
            tc.tile_pool(name="pp", bufs=2, space="PSUM") as pp,
            tc.tile_pool(name="tpp", bufs=4, space="PSUM") as tpp,
        ):
            # --- small constants / activations in first ---
            cst = ap_.tile([BL, NCONST], fp32, tag="cst")
            nc.sync.dma_start(cst[:], cst_t[:])
            ones_s = ap_.tile([1, BL], fp32, tag="ones")
            nc.sync.dma_start(ones_s[:], one_t[:])
            id_s = ap_.tile([BL, BL], fp32, tag="id8")
            nc.sync.dma_start(id_s[:], id_t[:])
            x_s = ap_.tile([BL, IN], fp32, tag="x")
            nc.sync.dma_start(x_s[:], x_t[:])
            xt_s = ap_.tile([P128, NKS[0] * BL], fp32, tag="xt")
            nc.sync.dma_start(xt_s[:], xt_t[:])
            b_s = []
            for l in range(3):
                t = ap_.tile([1, NOUTS[l]], fp32, tag=f"bh{l}")
                nc.sync.dma_start(t[:], b_t[l][:])
                b_s.append(t)
            m_s = [None]
            for l in (1, 2):
                t = ap_.tile([BL, NINS[l]], fp32, tag=f"m{l}")
                nc.sync.dma_start(t[:], m_t[l][:])
                m_s.append(t)
            m4_s = ap_.tile([BL, OUT], fp32, tag="m4")
            nc.sync.dma_start(m4_s[:], m4_t[:])

            # --- weights, chunked [128, nout] so matmuls chase the DMAs ---
            wcs = []
            for l in range(3):
                chunks = []
                for k in range(NKS[l]):
                    t = wp.tile([P128, NOUTS[l]], fp32, tag=f"w{l}c{k}")
                    nc.sync.dma_start(
                        t[:], w_t[l][:, k * NOUTS[l] : (k + 1) * NOUTS[l]]
                    )
                    chunks.append(t)
                wcs.append(chunks)

            def col(j):
                return cst[:, j : j + 1]

            def layer(l, vi, vt):
                nk, nin, nout = NKS[l], NINS[l], NOUTS[l]
                # row stats: s' = rowsum(vi'), q' = rowsum(vi'^2)
                s1 = scp.tile([BL, 1], fp32, tag=f"s{l}")
                q1 = scp.tile([BL, 1], fp32, tag=f"q{l}")
                sq = scp.tile([BL, nin], fp32, tag=f"sq{l}")
                nc.vector.reduce_sum(out=s1[:], in_=vi[:], axis=AX.X)
                nc.scalar.activation(
                    out=sq[:], in_=vi[:], func=AF.Square, accum_out=q1[:]
                )
                # alpha = C2*s' + k_alpha
                al = scp.tile([BL, 1], fp32, tag=f"al{l}")
                nc.vector.tensor_scalar(
                    al[:], s1[:], col(C_C2), col(C_KA0 + l), ALU.mult, ALU.add
                )
                # delta = C0*q' + k_delta + Cb*s' (+ 2*C0*cross)
                d1 = scp.tile([BL, 1], fp32, tag=f"d1{l}")
                d2 = scp.tile([BL, 1], fp32, tag=f"d2{l}")
                nc.vector.tensor_scalar(
                    d1[:], q1[:], col(C_C0), col(C_KD0 + l), ALU.mult, ALU.add
                )
                nc.vector.tensor_scalar(d2[:], s1[:], col(C_CB), None, ALU.mult)
                de = scp.tile([BL, 1], fp32, tag=f"de{l}")
                nc.vector.tensor_tensor(de[:], d1[:], d2[:], ALU.add)
                if m_s[l] is not None:
                    # (tensor_tensor_reduce traps on HW; use mult + reduce)
                    crs = scp.tile([BL, nin], fp32, tag=f"crs{l}")
                    cr = scp.tile([BL, 1], fp32, tag=f"cr{l}")
                    nc.vector.tensor_tensor(crs[:], vi[:], m_s[l][:], ALU.mult)
                    nc.vector.reduce_sum(out=cr[:], in_=crs[:], axis=AX.X)
                    c2t = scp.tile([BL, 1], fp32, tag=f"c2t{l}")
                    nc.vector.tensor_scalar(c2t[:], cr[:], col(C_2C0), None, ALU.mult)
                    de2 = scp.tile([BL, 1], fp32, tag=f"de2{l}")
                    nc.vector.tensor_tensor(de2[:], de[:], c2t[:], ALU.add)
                    de = de2
                # P = vi' @ W.T + bhat  (bias via K=1 ones x bhat matmul)
                Pt = pp.tile([BL, nout], fp32, tag="P")
                for k in range(nk):
                    nc.tensor.matmul(
                        Pt[:],
                        vt[:, k * BL : (k + 1) * BL],
                        wcs[l][k][:],
                        start=(k == 0),
                        stop=False,
                    )
                nc.tensor.matmul(Pt[:], ones_s[:], b_s[l][:], start=False, stop=True)
                # epilogue: out' = relu(P*alpha) + (C1*P + delta)   [alpha > 0]
                vja = ap_.tile([BL, nout], fp32, tag=f"vja{l}")
                nc.scalar.activation(
                    out=vja[:], in_=Pt[:], func=AF.Relu, scale=al[:, 0:1]
                )
                tC = ap_.tile([BL, nout], fp32, tag=f"tC{l}")
                nc.vector.tensor_scalar(
                    tC[:], Pt[:], col(C_C1), de[:, 0:1], ALU.mult, ALU.add
                )
                o = ap_.tile([BL, nout], fp32, tag=f"o{l}")
                nc.vector.tensor_tensor(o[:], vja[:], tC[:], ALU.add)
                return o

            def transpose_of(o, l, nout):
                vt = ap_.tile([P128, (nout // P128) * BL], fp32, tag=f"vt{l}")
                for c in range(nout // P128):
                    tp = tpp.tile([P128, BL], fp32, tag="tp")
                    nc.tensor.transpose(
                        tp[:], o[:, c * P128 : (c + 1) * P128], id_s[:]
                    )
                    nc.any.tensor_copy(out=vt[:, c * BL : (c + 1) * BL], in_=tp[:])
                return vt

            o1 = layer(0, x_s, xt_s)
            vt2 = transpose_of(o1, 1, H1)
            o2 = layer(1, o1, vt2)
            vt3 = transpose_of(o2, 2, H2)
            o3 = layer(2, o2, vt3)
            # out = out3' + m4
            of = ap_.tile([BL, OUT], fp32, tag="of")
            nc.vector.tensor_tensor(of[:], o3[:], m4_s[:], ALU.add)
            nc.sync.dma_start(out_t[:], of[:])

    nc.compile()
    return nc


def get_nc():
    if "nc" not in _NC_CACHE:
        _NC_CACHE["nc"] = _build_nc()
    return _NC_CACHE["nc"]


def _chunk_pt(a):
    """[R, C] -> [128, (R//128)*C]: row-chunks of 128 side by side."""
    r, c = a.shape
    nk = r // P128
    return np.ascontiguousarray(
        a.reshape(nk, P128, c).transpose(1, 0, 2).reshape(P128, nk * c)
    )


def host_prep(x, fc1_w, fc1_b, fc2_w, fc2_b, fc3_w, fc3_b,
              conv1_w, conv1_b, conv2_w, conv2_b, batch_num):
    f32 = np.float32
    x = np.asarray(x, f32)
    fc1_w = np.asarray(fc1_w, f32)
    fc2_w = np.asarray(fc2_w, f32)
    fc3_w = np.asarray(fc3_w, f32)
    fc1_b = np.asarray(fc1_b, f32)
    fc2_b = np.asarray(fc2_b, f32)
    fc3_b = np.asarray(fc3_b, f32)

    bn = float(np.asarray(batch_num).item())
    scale = RATE / bn
    coef = (np.asarray(conv2_w, np.float64) @ np.asarray(conv1_w, np.float64))[0]
    bc = float(
        (np.asarray(conv2_w, np.float64) @ np.asarray(conv1_b, np.float64))[0]
        + np.asarray(conv2_b, np.float64)[0]
    )
    C0, C1, C2 = (scale * coef).astype(np.float64)
    Cb = scale * bc

    m2 = (-C1 * fc1_b.astype(np.float64)).astype(f32)
    m3 = (-C1 * fc2_b.astype(np.float64)).astype(f32)
    m4 = (-C1 * fc3_b.astype(np.float64)).astype(f32)
    bh1 = fc1_b
    bh2 = (fc2_b + m2 @ fc2_w.T).astype(f32)
    bh3 = (fc3_b + m3 @ fc3_w.T).astype(f32)

    ka = [1.0, 1.0 + C2 * float(m2.sum()), 1.0 + C2 * float(m3.sum())]
    kd = [
        0.0,
        C0 * float(m2 @ m2) + Cb * float(m2.sum()),
        C0 * float(m3 @ m3) + Cb * float(m3.sum()),
    ]
    cvec = np.array([C0, C1, C2, Cb, 2 * C0] + ka + kd, dtype=f32)
    cst = np.broadcast_to(cvec, (BL, NCONST)).copy()

    common = {
        "w1t": _chunk_pt(fc1_w.T),
        "w2t": _chunk_pt(fc2_w.T),
        "w3t": _chunk_pt(fc3_w.T),
        "bh1": bh1.reshape(1, H1),
        "bh2": bh2.reshape(1, H2),
        "bh3": bh3.reshape(1, OUT),
        "m2r": np.broadcast_to(m2, (BL, H1)).copy(),
        "m3r": np.broadcast_to(m3, (BL, H2)).copy(),
        "m4r": np.broadcast_to(m4, (BL, OUT)).copy(),
        "cst": cst,
        "ones1": np.ones((1, BL), f32),
        "id8": np.eye(BL, dtype=f32),
    }
    common = {k: np.ascontiguousarray(v, dtype=f32) for k, v in common.items()}

    in_maps = []
    for k in range(NCORES):
        xk = np.ascontiguousarray(x[k * BL : (k + 1) * BL], dtype=f32)
        in_maps.append(dict(common, xb=xk, xtb=_chunk_pt(xk.T.copy())))
    return in_maps


def kernel(**inputs):
    from concourse.bass_utils import run_bass_kernel_spmd

    nc = get_nc()
    in_maps = host_prep(**inputs)
    res = run_bass_kernel_spmd(nc, in_maps, core_ids=list(range(NCORES)))
    out = np.concatenate([res.results[k]["outb"] for k in range(NCORES)], axis=0)
    return np.ascontiguousarray(out, dtype=np.float32)


# revision 8
# speedup vs baseline: 1.5212x; 1.5212x over previous
# Trainium2 Bass kernel for nn_DiffNet.
#
# Math: the conv2(conv1(.)) meta-MLP is affine per element, so with
#   coef = (conv2_w @ conv1_w)[0]  (c0, c1, c2),
#   bc   = (conv2_w @ conv1_b)[0] + conv2_b[0],
#   scale = RATE / batch_num,
# each layer (W, b) of the reference reduces to
#   z  = vi @ W.T                      (pre-bias matmul)
#   vj = relu(z + b)
#   s  = rowsum(vi),  q = rowsum(vi^2)
#   out = (1 + C2*s) * vj + C1*z + (C0*q + Cb*s)
# with C* = scale * (c*, bc).  No [B, out, in] tensor is ever materialized.
#
# Sharding: data-parallel over batch (64 rows -> 8 rows/core), weights
# replicated per core, zero collectives.
#
# Device-side bias folding: PSUM holds P = vi' @ W.T + bhat, where inputs are
# represented as vi = vi' + m (m a constant row vector, m1 = 0) and
# bhat = b + m @ W.T, so P = z + b exactly.  Then
#   out' = alpha (.) relu(P) + C1*P + delta,   out = out' - C1*b,
# so the next layer's constant offset is m_next = -C1*b, folded on host into
# bhat_next, k_alpha, k_delta, and the q cross-term.
#
# Matmul operands (weights, transposed activations, bias rows) are fp16:
# 4x faster PE streaming than fp32 and half the HBM traffic; accumulation
# and the whole epilogue stay fp32 (measured l2 rel err ~5e-4).

import numpy as np

RATE = 0.01
B, IN, H1, H2, OUT = 64, 1024, 512, 512, 256
NCORES = 8
BL = B // NCORES  # 8 rows per core
P128 = 128

# const-vector columns (in pk8)
C_C0, C_C1, C_C2, C_CB, C_2C0 = 0, 1, 2, 3, 4
C_KA0 = 5   # 5,6,7 = k_alpha per layer
C_KD0 = 8   # 8,9,10 = k_delta per layer
NCONST = 11

# pk1 (fp16, 1 partition): ones row | bhat1 | bhat2 | bhat3
PK1_ONES = 0
PK1_B = [8, 8 + H1, 8 + H1 + H2]
PK1_LEN = 8 + H1 + H2 + OUT

# pk8 (fp32, 8 partitions): x | m2r | m3r | m4r | cst | id8
PK8_X = 0
PK8_M = [None, IN, IN + H1]
PK8_M4 = IN + H1 + H2
PK8_CST = PK8_M4 + OUT
PK8_ID = PK8_CST + NCONST
PK8_LEN = PK8_ID + BL

# wall (fp16): w1 chunks | w2 chunks | w3 chunks
W_OFF = [0, (IN // P128) * H1, (IN // P128) * H1 + (H1 // P128) * H2]
W_LEN = W_OFF[2] + (H2 // P128) * OUT  # 7168

NKS = [IN // P128, H1 // P128, H2 // P128]
NINS = [IN, H1, H2]
NOUTS = [H1, H2, OUT]

_NC_CACHE = {}
DEBUG_TAPS = False


def _build_nc():
    import concourse.bacc as bacc
    import concourse.mybir as mybir
    import concourse.tile as tile

    fp32 = mybir.dt.float32
    fp16 = mybir.dt.float16
    AF = mybir.ActivationFunctionType
    ALU = mybir.AluOpType
    AX = mybir.AxisListType

    nc = bacc.Bacc("TRN2", target_bir_lowering=False, debug=False)

    pk1_t = nc.dram_tensor("pk1", [1, PK1_LEN], fp16, kind="ExternalInput")
    pk8_t = nc.dram_tensor("pk8", [BL, PK8_LEN], fp32, kind="ExternalInput")
    xt_t = nc.dram_tensor("xtb", [P128, NKS[0] * BL], fp16, kind="ExternalInput")
    w_t = nc.dram_tensor("wall", [P128, W_LEN], fp16, kind="ExternalInput")
    out_t = nc.dram_tensor("outb", [BL, OUT], fp32, kind="ExternalOutput")

    with tile.TileContext(nc) as tc:
        _dbg_taps = {}
        with (
            tc.tile_pool(name="wp", bufs=1) as wp,
            tc.tile_pool(name="actp", bufs=1) as ap_,
            tc.tile_pool(name="scp", bufs=1) as scp,
            tc.tile_pool(name="pp", bufs=2, space="PSUM") as pp,
            tc.tile_pool(name="tpp", bufs=4, space="PSUM") as tpp,
        ):
            pk1 = ap_.tile([1, PK1_LEN], fp16, tag="pk1")
            nc.sync.dma_start(pk1[:], pk1_t[:])
            pk8 = ap_.tile([BL, PK8_LEN], fp32, tag="pk8")
            nc.sync.dma_start(pk8[:], pk8_t[:])
            xt_s = ap_.tile([P128, NKS[0] * BL], fp16, tag="xt")
            nc.sync.dma_start(xt_s[:], xt_t[:])
            # weights: w1 in two halves, then w2, w3 — pipelines with L1
            wseg = []  # (tile, col offset within wall)
            for name, lo, hi in (
                ("w1a", 0, 2048),
                ("w1b", 2048, 4096),
                ("w2", W_OFF[1], W_OFF[1] + 2048),
                ("w3", W_OFF[2], W_OFF[2] + 1024),
            ):
                t = wp.tile([P128, hi - lo], fp16, tag=name)
                nc.sync.dma_start(t[:], w_t[:, lo:hi])
                wseg.append((t, lo))

            def wslice(l, k):
                off = W_OFF[l] + k * NOUTS[l]
                for t, lo in wseg:
                    if lo <= off and off + NOUTS[l] <= lo + t.shape[1]:
                        return t[:, off - lo : off - lo + NOUTS[l]]
                raise AssertionError("bad weight slice")

            x_s = pk8[:, PK8_X : PK8_X + IN]
            id_s = pk8[:, PK8_ID : PK8_ID + BL]

            def col(j):
                c = PK8_CST + j
                return pk8[:, c : c + 1]

            # vt[l][k]: fp16 [128, BL] lhsT chunk tiles per layer
            vt = [[xt_s[:, k * BL : (k + 1) * BL] for k in range(NKS[0])]]
            o_prev = [None]

            def layer(l, s1, q1, cr1):
                nk, nout = NKS[l], NOUTS[l]
                # alpha = C2*s + k_alpha ; delta = C0*q + k_delta + Cb*s (+2C0*cr)
                al = scp.tile([BL, 1], fp32, tag=f"al{l}")
                nc.vector.tensor_scalar(
                    al[:], s1, col(C_C2), col(C_KA0 + l), ALU.mult, ALU.add
                )
                d1 = scp.tile([BL, 1], fp32, tag=f"d1{l}")
                d2 = scp.tile([BL, 1], fp32, tag=f"d2{l}")
                nc.vector.tensor_scalar(
                    d1[:], q1, col(C_C0), col(C_KD0 + l), ALU.mult, ALU.add
                )
                nc.vector.tensor_scalar(d2[:], s1, col(C_CB), None, ALU.mult)
                de = scp.tile([BL, 1], fp32, tag=f"de{l}")
                nc.vector.tensor_tensor(de[:], d1[:], d2[:], ALU.add)
                if cr1 is not None:
                    c2t = scp.tile([BL, 1], fp32, tag=f"c2t{l}")
                    nc.vector.tensor_scalar(c2t[:], cr1, col(C_2C0), None, ALU.mult)
                    de2 = scp.tile([BL, 1], fp32, tag=f"de2{l}")
                    nc.vector.tensor_tensor(de2[:], de[:], c2t[:], ALU.add)
                    de = de2
                # P = vi' @ W.T + bhat
                Pt = pp.tile([BL, nout], fp32, tag="P")
                for k in range(nk):
                    nc.tensor.matmul(
                        Pt[:], vt[l][k], wslice(l, k), start=(k == 0), stop=False
                    )
                boff = PK1_B[l]
                nc.tensor.matmul(
                    Pt[:],
                    pk1[:, PK1_ONES : PK1_ONES + BL],
                    pk1[:, boff : boff + nout],
                    start=False,
                    stop=True,
                )
                # epilogue: out' = relu(P*alpha) + (C1*P + delta)   [alpha > 0]
                # (accum_out must not be combined with a second scalar op:
                #  the reduction takes the ALU stage and op1 is dropped)
                vja = ap_.tile([BL, nout], fp32, tag=f"vja{l}")
                nc.scalar.activation(
                    out=vja[:], in_=Pt[:], func=AF.Relu, scale=al[:, 0:1]
                )
                tC = ap_.tile([BL, nout], fp32, tag=f"tC{l}")
                nc.vector.tensor_scalar(
                    tC[:], Pt[:], col(C_C1), de[:, 0:1], ALU.mult, ALU.add
                )
                o = ap_.tile([BL, nout], fp32, tag=f"o{l}")
                nc.vector.tensor_tensor(o[:], vja[:], tC[:], ALU.add)
                if DEBUG_TAPS and l == 0:
                    pc = ap_.tile([BL, nout], fp32, tag="dbgP")
                    nc.vector.tensor_copy(pc[:], Pt[:])
                    _dbg_taps.update({
                        "dbg_P1": pc[:], "dbg_al1": al[:], "dbg_de1": de[:],
                        "dbg_vja1": vja[:], "dbg_tC1": tC[:],
                    })
                if l < 2:
                    # next-layer stats
                    s_n = scp.tile([BL, 1], fp32, tag=f"s{l + 1}")
                    nc.vector.reduce_sum(out=s_n[:], in_=o[:], axis=AX.X)
                    q_n = scp.tile([BL, 1], fp32, tag=f"q{l + 1}")
                    sq = scp.tile([BL, nout], fp32, tag=f"sq{l + 1}")
                    nc.scalar.activation(
                        out=sq[:], in_=o[:], func=AF.Square, accum_out=q_n[:]
                    )
                    m_sl = pk8[:, PK8_M[l + 1] : PK8_M[l + 1] + nout]
                    crs = scp.tile([BL, nout], fp32, tag=f"crs{l + 1}")
                    cr_n = scp.tile([BL, 1], fp32, tag=f"cr{l + 1}")
                    nc.vector.tensor_tensor(crs[:], o[:], m_sl, ALU.mult)
                    nc.vector.reduce_sum(out=cr_n[:], in_=crs[:], axis=AX.X)
                    # transposes -> next layer's fp16 lhsT chunks
                    nxt = []
                    for c in range(nout // P128):
                        tp = tpp.tile([P128, BL], fp32, tag="tp")
                        nc.tensor.transpose(
                            tp[:], o[:, c * P128 : (c + 1) * P128], id_s
                        )
                        v = ap_.tile([P128, BL], fp16, tag=f"vt{l + 1}_{c}")
                        nc.any.tensor_copy(out=v[:], in_=tp[:])
                        nxt.append(v[:])
                    vt.append(nxt)
                    return o, s_n[:, 0:1], q_n[:, 0:1], cr_n[:, 0:1]
                return o, None, None, None

            # layer-1 stats straight from fp32 x
            s1 = scp.tile([BL, 1], fp32, tag="s0")
            nc.vector.reduce_sum(out=s1[:], in_=x_s, axis=AX.X)
            q1 = scp.tile([BL, 1], fp32, tag="q0")
            sq0 = scp.tile([BL, IN], fp32, tag="sq0")
            nc.scalar.activation(out=sq0[:], in_=x_s, func=AF.Square, accum_out=q1[:])

            o1, s2, q2, cr2 = layer(0, s1[:, 0:1], q1[:, 0:1], None)
            o2, s3, q3, cr3 = layer(1, s2, q2, cr2)
            o3, _, _, _ = layer(2, s3, q3, cr3)

            of = ap_.tile([BL, OUT], fp32, tag="of")
            nc.vector.tensor_tensor(
                of[:], o3[:], pk8[:, PK8_M4 : PK8_M4 + OUT], ALU.add
            )
            nc.sync.dma_start(out_t[:], of[:])

            if DEBUG_TAPS:
                taps = {
                    "dbg_o1": o1[:],
                    "dbg_s2": s2,
                    "dbg_q2": q2,
                    "dbg_cr2": cr2,
                    "dbg_vt2_0": vt[1][0],
                    "dbg_o2": o2[:],
                    "dbg_s1": s1[:],
                    "dbg_q1": q1[:],
                }
                taps.update(_dbg_taps)
                for name, ap in taps.items():
                    t = nc.dram_tensor(
                        name, list(ap.shape), ap.dtype, kind="ExternalOutput"
                    )
                    nc.sync.dma_start(t[:], ap)

    nc.compile()
    return nc


def get_nc():
    if "nc" not in _NC_CACHE:
        _NC_CACHE["nc"] = _build_nc()
    return _NC_CACHE["nc"]


def _chunk_pt(a, dtype):
    """[R, C] -> [128, (R//128)*C]: row-chunks of 128 side by side."""
    r, c = a.shape
    nk = r // P128
    return np.ascontiguousarray(
        a.reshape(nk, P128, c).transpose(1, 0, 2).reshape(P128, nk * c), dtype=dtype
    )


def host_prep(x, fc1_w, fc1_b, fc2_w, fc2_b, fc3_w, fc3_b,
              conv1_w, conv1_b, conv2_w, conv2_b, batch_num):
    f32, f16 = np.float32, np.float16
    x = np.asarray(x, f32)
    fc1_w = np.asarray(fc1_w, f32)
    fc2_w = np.asarray(fc2_w, f32)
    fc3_w = np.asarray(fc3_w, f32)
    fc1_b = np.asarray(fc1_b, f32)
    fc2_b = np.asarray(fc2_b, f32)
    fc3_b = np.asarray(fc3_b, f32)

    bn = float(np.asarray(batch_num).item())
    scale = RATE / bn
    coef = (np.asarray(conv2_w, np.float64) @ np.asarray(conv1_w, np.float64))[0]
    bc = float(
        (np.asarray(conv2_w, np.float64) @ np.asarray(conv1_b, np.float64))[0]
        + np.asarray(conv2_b, np.float64)[0]
    )
    C0, C1, C2 = (scale * coef).astype(np.float64)
    Cb = scale * bc

    m2 = (-C1 * fc1_b.astype(np.float64)).astype(f32)
    m3 = (-C1 * fc2_b.astype(np.float64)).astype(f32)
    m4 = (-C1 * fc3_b.astype(np.float64)).astype(f32)
    bh1 = fc1_b
    bh2 = (fc2_b + m2 @ fc2_w.T).astype(f32)
    bh3 = (fc3_b + m3 @ fc3_w.T).astype(f32)

    ka = [1.0, 1.0 + C2 * float(m2.sum()), 1.0 + C2 * float(m3.sum())]
    kd = [
        0.0,
        C0 * float(m2 @ m2) + Cb * float(m2.sum()),
        C0 * float(m3 @ m3) + Cb * float(m3.sum()),
    ]
    cvec = np.array([C0, C1, C2, Cb, 2 * C0] + ka + kd, dtype=f32)

    pk1 = np.zeros((1, PK1_LEN), f16)
    pk1[0, PK1_ONES : PK1_ONES + BL] = 1.0
    pk1[0, PK1_B[0] : PK1_B[0] + H1] = bh1.astype(f16)
    pk1[0, PK1_B[1] : PK1_B[1] + H2] = bh2.astype(f16)
    pk1[0, PK1_B[2] : PK1_B[2] + OUT] = bh3.astype(f16)

    wall = np.empty((P128, W_LEN), f16)
    wall[:, W_OFF[0] : W_OFF[1]] = _chunk_pt(fc1_w.T, f16)
    wall[:, W_OFF[1] : W_OFF[2]] = _chunk_pt(fc2_w.T, f16)
    wall[:, W_OFF[2] : W_LEN] = _chunk_pt(fc3_w.T, f16)

    pk8_base = np.zeros((BL, PK8_LEN), f32)
    pk8_base[:, PK8_M[1] : PK8_M[1] + H1] = m2
    pk8_base[:, PK8_M[2] : PK8_M[2] + H2] = m3
    pk8_base[:, PK8_M4 : PK8_M4 + OUT] = m4
    pk8_base[:, PK8_CST : PK8_CST + NCONST] = cvec
    pk8_base[:, PK8_ID : PK8_ID + BL] = np.eye(BL, dtype=f32)

    common = {
        "pk1": np.ascontiguousarray(pk1),
        "wall": np.ascontiguousarray(wall),
    }
    in_maps = []
    for k in range(NCORES):
        xk = np.ascontiguousarray(x[k * BL : (k + 1) * BL], dtype=f32)
        pk8 = pk8_base.copy()
        pk8[:, PK8_X : PK8_X + IN] = xk
        in_maps.append(
            dict(common, pk8=pk8, xtb=_chunk_pt(xk.T.copy(), f16))
        )
    return in_maps


def kernel(**inputs):
    from concourse.bass_utils import run_bass_kernel_spmd

    nc = get_nc()
    in_maps = host_prep(**inputs)
    res = run_bass_kernel_spmd(nc, in_maps, core_ids=list(range(NCORES)))
    out = np.concatenate([res.results[k]["outb"] for k in range(NCORES)], axis=0)
    return np.ascontiguousarray(out, dtype=np.float32)


# revision 9
# speedup vs baseline: 1.5865x; 1.0429x over previous
# Trainium2 Bass kernel for nn_DiffNet.
#
# Math: the conv2(conv1(.)) meta-MLP is affine per element, so with
#   coef = (conv2_w @ conv1_w)[0]  (c0, c1, c2),
#   bc   = (conv2_w @ conv1_b)[0] + conv2_b[0],
#   scale = RATE / batch_num,
# each layer (W, b) of the reference reduces to
#   z  = vi @ W.T                      (pre-bias matmul)
#   vj = relu(z + b)
#   s  = rowsum(vi),  q = rowsum(vi^2)
#   out = (1 + C2*s) * vj + C1*z + (C0*q + Cb*s)
# with C* = scale * (c*, bc).  No [B, out, in] tensor is ever materialized.
#
# Sharding: data-parallel over batch (64 rows -> 8 rows/core), weights
# replicated per core, zero collectives.
#
# Device-side bias folding: PSUM holds P = vi' @ W.T + bhat, where inputs are
# represented as vi = vi' + m (m a constant row vector, m1 = 0) and
# bhat = b + m @ W.T, so P = z + b exactly.  Then
#   out' = alpha (.) relu(P) + C1*P + delta,   out = out' - C1*b,
# so the next layer's constant offset is m_next = -C1*b, folded on host into
# bhat_next, k_alpha, k_delta, and the q cross-term.
#
# Matmul operands are fp16 (4x PE rate vs fp32, half the HBM bytes);
# accumulation + epilogue stay fp32 (measured l2 rel err ~5e-4).
#
# Perf notes (from HW traces):
# - HWDGE descriptor-gen paces a queue at ~desc_size/20ns; per-partition
#   runs must be >=4KB, so all fp16 operands live in ONE [128, 7232] pack
#   (xt | w1 | w2 | w3) DMA'd in 4 column-slices on the sync queue while
#   pk1/pk8 ride the scalar queue.
# - PE HAM clock-gate: ~4us of warm-up matmuls on junk tiles first, so the
#   real matmuls run at 2.4GHz instead of 1.2.
# - Kernel tail pays ~115ns per semaphore reset: keep instruction count low
#   (fused delta reduction, single transpose-copy per boundary).

import numpy as np

RATE = 0.01
B, IN, H1, H2, OUT = 64, 1024, 512, 512, 256
NCORES = 8
BL = B // NCORES  # 8 rows per core
P128 = 128

# const columns in pk8: scalars, then per-layer [Cb, C0, 2C0] triples
C_C1, C_C2 = 0, 1
C_KA0 = 2    # 2,3,4 = k_alpha per layer
C_KD0 = 5    # 5,6,7 = k_delta per layer
C_ZERO = 8
C_TRI0 = 9   # 9..17: per-layer [Cb, C0, twoC0] (twoC0 = 0 for layer 0)
NCONST = 18

# pk1 (fp16, 1 partition): ones row | bhat1 | bhat2 | bhat3
PK1_ONES = 0
PK1_B = [8, 8 + H1, 8 + H1 + H2]
PK1_LEN = 8 + H1 + H2 + OUT

# pk8 (fp32, 8 partitions): x | m2r | m3r | m4r | cst | id8
PK8_X = 0
PK8_M = [None, IN, IN + H1]
PK8_M4 = IN + H1 + H2
PK8_CST = PK8_M4 + OUT
PK8_ID = PK8_CST + NCONST
PK8_LEN = PK8_ID + BL

# wall (fp16): xt | w1 chunks | w2 chunks | w3 chunks
XT_OFF = 0
XT_LEN = (IN // P128) * BL  # 64
W_OFF = [XT_LEN, XT_LEN + 4096, XT_LEN + 6144]
W_LEN = XT_LEN + 7168  # 7232

NKS = [IN // P128, H1 // P128, H2 // P128]
NOUTS = [H1, H2, OUT]

N_WARMUP = 7  # PE clock-gate warmup matmuls

_NC_CACHE = {}
DEBUG_TAPS = False


def _build_nc():
    import concourse.bacc as bacc
    import concourse.mybir as mybir
    import concourse.tile as tile

    fp32 = mybir.dt.float32
    fp16 = mybir.dt.float16
    AF = mybir.ActivationFunctionType
    ALU = mybir.AluOpType
    AX = mybir.AxisListType

    nc = bacc.Bacc("TRN2", target_bir_lowering=False, debug=False)

    pk1_t = nc.dram_tensor("pk1", [1, PK1_LEN], fp16, kind="ExternalInput")
    pk8_t = nc.dram_tensor("pk8", [BL, PK8_LEN], fp32, kind="ExternalInput")
    w_t = nc.dram_tensor("wall", [P128, W_LEN], fp16, kind="ExternalInput")
    out_t = nc.dram_tensor("outb", [BL, OUT], fp32, kind="ExternalOutput")

    with tile.TileContext(nc) as tc:
        with (
            tc.tile_pool(name="wp", bufs=1) as wp,
            tc.tile_pool(name="actp", bufs=1) as ap_,
            tc.tile_pool(name="scp", bufs=1) as scp,
            tc.tile_pool(name="pp", bufs=2, space="PSUM") as pp,
            tc.tile_pool(name="tpp", bufs=2, space="PSUM") as tpp,
        ):
            # --- PE warm-up: junk matmuls release the HAM clock gate ---
            junk_a = wp.tile([P128, BL], fp16, tag="junk_a")
            junk_w = wp.tile([P128, 512], fp16, tag="junk_w")
            nc.vector.memset(junk_a[:], 0.0)
            nc.vector.memset(junk_w[:], 0.0)
            warm_p = pp.tile([BL, 512], fp32, tag="warm")
            for _ in range(N_WARMUP):
                nc.tensor.matmul(
                    warm_p[:], junk_a[:, :BL], junk_w[:], start=True, stop=True
                )

            # --- DMAs: weights on sync queue, packs on scalar queue ---
            wseg = []  # (tile, col offset within wall)
            for name, lo, hi in (
                ("wA", 0, 2112),          # xt + first half of w1
                ("wB", 2112, 4160),       # rest of w1
                ("wC", 4160, 6208),       # w2
                ("wD", 6208, 7232),       # w3
            ):
                t = wp.tile([P128, hi - lo], fp16, tag=name)
                nc.sync.dma_start(t[:], w_t[:, lo:hi])
                wseg.append((t, lo))
            pk1 = ap_.tile([1, PK1_LEN], fp16, tag="pk1")
            nc.scalar.dma_start(pk1[:], pk1_t[:])
            pk8 = ap_.tile([BL, PK8_LEN], fp32, tag="pk8")
            nc.scalar.dma_start(pk8[:], pk8_t[:])

            def wall_slice(lo, n):
                for t, off in wseg:
                    if off <= lo and lo + n <= off + t.shape[1]:
                        return t[:, lo - off : lo - off + n]
                raise AssertionError("bad wall slice")

            x_s = pk8[:, PK8_X : PK8_X + IN]
            id_s = pk8[:, PK8_ID : PK8_ID + BL]

            def col(j):
                c = PK8_CST + j
                return pk8[:, c : c + 1]

            # lhsT chunk slices per layer (fp16 [128, BL] each)
            vt = [[wall_slice(XT_OFF + k * BL, BL) for k in range(NKS[0])]]

            def layer(l, svec):
                """svec: [BL, 3] tile, cols = s | q | cross (cross only l>0)."""
                nk, nout = NKS[l], NOUTS[l]
                w = 2 if l == 0 else 3
                # alpha = C2*s + k_alpha
                al = scp.tile([BL, 1], fp32, tag=f"al{l}")
                nc.vector.tensor_scalar(
                    al[:], svec[:, 0:1], col(C_C2), col(C_KA0 + l), ALU.mult, ALU.add
                )
                # delta = sum(svec * [Cb, C0, 2C0]) + k_delta
                dprod = scp.tile([BL, 3], fp32, tag=f"dp{l}")
                nc.vector.tensor_tensor(
                    dprod[:, :w],
                    svec[:, :w],
                    pk8[:, PK8_CST + C_TRI0 + 3 * l : PK8_CST + C_TRI0 + 3 * l + w],
                    ALU.mult,
                )
                de = scp.tile([BL, 1], fp32, tag=f"de{l}")
                nc.vector.tensor_reduce(
                    out=de[:], in_=dprod[:, :w], axis=AX.X, op=ALU.add
                )
                de2 = scp.tile([BL, 1], fp32, tag=f"de2{l}")
                nc.vector.tensor_scalar(
                    de2[:], de[:], col(C_KD0 + l), None, ALU.add
                )
                # P = vi' @ W.T + bhat
                Pt = pp.tile([BL, nout], fp32, tag="P")
                for k in range(nk):
                    nc.tensor.matmul(
                        Pt[:],
                        vt[l][k],
                        wall_slice(W_OFF[l] + k * nout, nout),
                        start=(k == 0),
                        stop=False,
                    )
                boff = PK1_B[l]
                nc.tensor.matmul(
                    Pt[:],
                    pk1[:, PK1_ONES : PK1_ONES + BL],
                    pk1[:, boff : boff + nout],
                    start=False,
                    stop=True,
                )
                # epilogue: out' = relu(P*alpha) + (C1*P + delta)   [alpha > 0]
                vja = ap_.tile([BL, nout], fp32, tag=f"vja{l}")
                nc.scalar.activation(
                    out=vja[:], in_=Pt[:], func=AF.Relu, scale=al[:, 0:1],
                    bias=col(C_ZERO),
                )
                tC = ap_.tile([BL, nout], fp32, tag=f"tC{l}")
                nc.vector.tensor_scalar(
                    tC[:], Pt[:], col(C_C1), de2[:, 0:1], ALU.mult, ALU.add
                )
                o = ap_.tile([BL, nout], fp32, tag=f"o{l}")
                nc.vector.tensor_tensor(o[:], vja[:], tC[:], ALU.add)
                if l == 2:
                    return o, None
                # transposes -> next layer's fp16 lhsT chunks (one copy)
                nch = nout // P128
                tp = tpp.tile([P128, nch * BL], fp32, tag="tp")
                for c in range(nch):
                    nc.tensor.transpose(
                        tp[:, c * BL : (c + 1) * BL],
                        o[:, c * P128 : (c + 1) * P128],
                        id_s,
                    )
                vtn = ap_.tile([P128, nch * BL], fp16, tag=f"vt{l + 1}")
                nc.any.tensor_copy(out=vtn[:], in_=tp[:])
                vt.append([vtn[:, k * BL : (k + 1) * BL] for k in range(nch)])
                # next-layer stats into svec columns
                sv = scp.tile([BL, 3], fp32, tag=f"sv{l + 1}")
                nc.vector.reduce_sum(out=sv[:, 0:1], in_=o[:], axis=AX.X)
                sq = scp.tile([BL, nout], fp32, tag=f"sq{l + 1}")
                nc.scalar.activation(
                    out=sq[:], in_=o[:], func=AF.Square, accum_out=sv[:, 1:2]
                )
                crs = scp.tile([BL, nout], fp32, tag=f"crs{l + 1}")
                nc.vector.tensor_tensor(
                    crs[:], o[:], pk8[:, PK8_M[l + 1] : PK8_M[l + 1] + nout], ALU.mult
                )
                nc.vector.reduce_sum(out=sv[:, 2:3], in_=crs[:], axis=AX.X)
                return o, sv

            # layer-1 stats straight from fp32 x
            sv1 = scp.tile([BL, 3], fp32, tag="sv1")
            nc.vector.reduce_sum(out=sv1[:, 0:1], in_=x_s, axis=AX.X)
            sq0 = scp.tile([BL, IN], fp32, tag="sq0")
            nc.scalar.activation(
                out=sq0[:], in_=x_s, func=AF.Square, accum_out=sv1[:, 1:2]
            )

            o1, sv2 = layer(0, sv1)
            o2, sv3 = layer(1, sv2)
            o3, _ = layer(2, sv3)

            of = ap_.tile([BL, OUT], fp32, tag="of")
            nc.vector.tensor_tensor(
                of[:], o3[:], pk8[:, PK8_M4 : PK8_M4 + OUT], ALU.add
            )
            nc.sync.dma_start(out_t[:], of[:])

            if DEBUG_TAPS:
                for name, ap in (("dbg_o1", o1[:]), ("dbg_o2", o2[:])):
                    t = nc.dram_tensor(
                        name, list(ap.shape), ap.dtype, kind="ExternalOutput"
                    )
                    nc.sync.dma_start(t[:], ap)

    nc.compile()
    return nc


def get_nc():
    if "nc" not in _NC_CACHE:
        _NC_CACHE["nc"] = _build_nc()
    return _NC_CACHE["nc"]


def _chunk_pt(a, dtype):
    """[R, C] -> [128, (R//128)*C]: row-chunks of 128 side by side."""
    r, c = a.shape
    nk = r // P128
    return np.ascontiguousarray(
        a.reshape(nk, P128, c).transpose(1, 0, 2).reshape(P128, nk * c), dtype=dtype
    )


def host_prep(x, fc1_w, fc1_b, fc2_w, fc2_b, fc3_w, fc3_b,
              conv1_w, conv1_b, conv2_w, conv2_b, batch_num):
    f32, f16 = np.float32, np.float16
    x = np.asarray(x, f32)
    fc1_w = np.asarray(fc1_w, f32)
    fc2_w = np.asarray(fc2_w, f32)
    fc3_w = np.asarray(fc3_w, f32)
    fc1_b = np.asarray(fc1_b, f32)
    fc2_b = np.asarray(fc2_b, f32)
    fc3_b = np.asarray(fc3_b, f32)

    bn = float(np.asarray(batch_num).item())
    scale = RATE / bn
    coef = (np.asarray(conv2_w, np.float64) @ np.asarray(conv1_w, np.float64))[0]
    bc = float(
        (np.asarray(conv2_w, np.float64) @ np.asarray(conv1_b, np.float64))[0]
        + np.asarray(conv2_b, np.float64)[0]
    )
    C0, C1, C2 = (scale * coef).astype(np.float64)
    Cb = scale * bc

    m2 = (-C1 * fc1_b.astype(np.float64)).astype(f32)
    m3 = (-C1 * fc2_b.astype(np.float64)).astype(f32)
    m4 = (-C1 * fc3_b.astype(np.float64)).astype(f32)
    bh1 = fc1_b
    bh2 = (fc2_b + m2 @ fc2_w.T).astype(f32)
    bh3 = (fc3_b + m3 @ fc3_w.T).astype(f32)

    ka = [1.0, 1.0 + C2 * float(m2.sum()), 1.0 + C2 * float(m3.sum())]
    kd = [
        0.0,
        C0 * float(m2 @ m2) + Cb * float(m2.sum()),
        C0 * float(m3 @ m3) + Cb * float(m3.sum()),
    ]
    cvec = np.zeros(NCONST, dtype=f32)
    cvec[C_C1], cvec[C_C2] = C1, C2
    cvec[C_KA0 : C_KA0 + 3] = ka
    cvec[C_KD0 : C_KD0 + 3] = kd
    for l in range(3):
        cvec[C_TRI0 + 3 * l : C_TRI0 + 3 * l + 3] = [
            Cb, C0, 0.0 if l == 0 else 2 * C0
        ]

    pk1 = np.zeros((1, PK1_LEN), f16)
    pk1[0, PK1_ONES : PK1_ONES + BL] = 1.0
    pk1[0, PK1_B[0] : PK1_B[0] + H1] = bh1.astype(f16)
    pk1[0, PK1_B[1] : PK1_B[1] + H2] = bh2.astype(f16)
    pk1[0, PK1_B[2] : PK1_B[2] + OUT] = bh3.astype(f16)

    wall_base = np.empty((P128, W_LEN), f16)
    wall_base[:, W_OFF[0] : W_OFF[0] + 4096] = _chunk_pt(fc1_w.T, f16)
    wall_base[:, W_OFF[1] : W_OFF[1] + 2048] = _chunk_pt(fc2_w.T, f16)
    wall_base[:, W_OFF[2] : W_OFF[2] + 1024] = _chunk_pt(fc3_w.T, f16)

    pk8_base = np.zeros((BL, PK8_LEN), f32)
    pk8_base[:, PK8_M[1] : PK8_M[1] + H1] = m2
    pk8_base[:, PK8_M[2] : PK8_M[2] + H2] = m3
    pk8_base[:, PK8_M4 : PK8_M4 + OUT] = m4
    pk8_base[:, PK8_CST : PK8_CST + NCONST] = cvec
    pk8_base[:, PK8_ID : PK8_ID + BL] = np.eye(BL, dtype=f32)

    in_maps = []
    for k in range(NCORES):
        xk = np.ascontiguousarray(x[k * BL : (k + 1) * BL], dtype=f32)
        pk8 = pk8_base.copy()
        pk8[:, PK8_X : PK8_X + IN] = xk
        wall = wall_base.copy()
        wall[:, XT_OFF : XT_OFF + XT_LEN] = _chunk_pt(xk.T.copy(), f16)
        in_maps.append({"pk1": pk1, "pk8": pk8, "wall": wall})
    return in_maps


def kernel(**inputs):
    from concourse.bass_utils import run_bass_kernel_spmd

    nc = get_nc()
    in_maps = host_prep(**inputs)
    res = run_bass_kernel_spmd(nc, in_maps, core_ids=list(range(NCORES)))
    out = np.concatenate([res.results[k]["outb"] for k in range(NCORES)], axis=0)
    return np.ascontiguousarray(out, dtype=np.float32)


# revision 10
# speedup vs baseline: 1.6217x; 1.0222x over previous
# Trainium2 Bass kernel for nn_DiffNet.
#
# Math: the conv2(conv1(.)) meta-MLP is affine per element, so with
#   coef = (conv2_w @ conv1_w)[0]  (c0, c1, c2),
#   bc   = (conv2_w @ conv1_b)[0] + conv2_b[0],
#   scale = RATE / batch_num,
# each layer (W, b) of the reference reduces to
#   z  = vi @ W.T                      (pre-bias matmul)
#   vj = relu(z + b)
#   s  = rowsum(vi),  q = rowsum(vi^2)
#   out = (1 + C2*s) * vj + C1*z + (C0*q + Cb*s)
# with C* = scale * (c*, bc).  No [B, out, in] tensor is ever materialized.
#
# Sharding: data-parallel over batch (64 rows -> 8 rows/core), weights
# replicated per core, zero collectives.
#
# Device-side bias folding: PSUM holds P = vi' @ W.T + bhat, where inputs are
# represented as vi = vi' + m (m a constant row vector, m1 = 0) and
# bhat = b + m @ W.T, so P = z + b exactly.  Then
#   out' = alpha (.) relu(P) + C1*P + delta,   out = out' - C1*b,
# so the next layer's constant offset is m_next = -C1*b, folded on host into
# bhat_next, k_alpha, k_delta, and the q cross-term.
#
# Matmul operands are fp16 (4x PE rate vs fp32, half the HBM bytes);
# accumulation + epilogue stay fp32 (measured l2 rel err ~5e-4).
#
# Perf notes (from HW traces):
# - HWDGE descriptor-gen paces a queue at ~desc_size/20ns; per-partition
#   runs must be >=4KB, so all fp16 operands live in ONE [128, 7232] pack
#   (xt | w1 | w2 | w3) DMA'd in 4 column-slices on the sync queue while
#   pk1/pk8 ride the scalar queue.
# - PE HAM clock-gate: ~4us of warm-up matmuls on junk tiles first, so the
#   real matmuls run at 2.4GHz instead of 1.2.
# - Kernel tail pays ~115ns per semaphore reset: keep instruction count low
#   (fused delta reduction, single transpose-copy per boundary).

import numpy as np

RATE = 0.01
B, IN, H1, H2, OUT = 64, 1024, 512, 512, 256
NCORES = 8
BL = B // NCORES  # 8 rows per core
P128 = 128

# const columns in pk8: scalars, then per-layer [Cb, C0, 2C0] triples
C_C1, C_C2 = 0, 1
C_KA0 = 2    # 2,3,4 = k_alpha per layer
C_KD0 = 5    # 5,6,7 = k_delta per layer
C_ZERO = 8
C_TRI0 = 9   # 9..17: per-layer [Cb, C0, twoC0] (twoC0 = 0 for layer 0)
NCONST = 18

# pk1 (fp16, 1 partition): ones row | bhat1 | bhat2 | bhat3
PK1_ONES = 0
PK1_B = [8, 8 + H1, 8 + H1 + H2]
PK1_LEN = 8 + H1 + H2 + OUT

# pk8 (fp32, 8 partitions): x | m2r | m3r | m4r | cst | id8
PK8_X = 0
PK8_M = [None, IN, IN + H1]
PK8_M4 = IN + H1 + H2
PK8_CST = PK8_M4 + OUT
PK8_ID = PK8_CST + NCONST
PK8_LEN = PK8_ID + BL

# wall (fp16): xt | w1 chunks | w2 chunks | w3 chunks
XT_OFF = 0
XT_LEN = (IN // P128) * BL  # 64
W_OFF = [XT_LEN, XT_LEN + 4096, XT_LEN + 6144]
W_LEN = XT_LEN + 7168  # 7232

NKS = [IN // P128, H1 // P128, H2 // P128]
NOUTS = [H1, H2, OUT]

N_WARMUP = 7  # PE clock-gate warmup matmuls

_NC_CACHE = {}
DEBUG_TAPS = False


def _build_nc():
    import concourse.bacc as bacc
    import concourse.mybir as mybir
    import concourse.tile as tile

    fp32 = mybir.dt.float32
    fp16 = mybir.dt.float16
    AF = mybir.ActivationFunctionType
    ALU = mybir.AluOpType
    AX = mybir.AxisListType

    nc = bacc.Bacc("TRN2", target_bir_lowering=False, debug=False)

    pk1_t = nc.dram_tensor("pk1", [1, PK1_LEN], fp16, kind="ExternalInput")
    pk8_t = nc.dram_tensor("pk8", [BL, PK8_LEN], fp32, kind="ExternalInput")
    w_t = nc.dram_tensor("wall", [P128, W_LEN], fp16, kind="ExternalInput")
    out_t = nc.dram_tensor("outb", [BL, OUT], fp32, kind="ExternalOutput")

    with tile.TileContext(nc) as tc:
        with (
            tc.tile_pool(name="wp", bufs=1) as wp,
            tc.tile_pool(name="actp", bufs=1) as ap_,
            tc.tile_pool(name="scp", bufs=1) as scp,
            tc.tile_pool(name="pp", bufs=2, space="PSUM") as pp,
            tc.tile_pool(name="tpp", bufs=2, space="PSUM") as tpp,
        ):
            # --- PE warm-up: junk matmuls release the HAM clock gate ---
            junk_a = wp.tile([P128, BL], fp16, tag="junk_a")
            junk_w = wp.tile([P128, 512], fp16, tag="junk_w")
            nc.vector.memset(junk_a[:], 0.0)
            nc.vector.memset(junk_w[:], 0.0)
            warm_p = pp.tile([BL, 512], fp32, tag="warm")
            for _ in range(N_WARMUP):
                nc.tensor.matmul(
                    warm_p[:], junk_a[:, :BL], junk_w[:], start=True, stop=True
                )

            # --- DMAs: all on the sync queue, small packs first ---
            pk1 = ap_.tile([1, PK1_LEN], fp16, tag="pk1")
            nc.sync.dma_start(pk1[:], pk1_t[:])
            pk8 = ap_.tile([BL, PK8_LEN], fp32, tag="pk8")
            nc.sync.dma_start(pk8[:], pk8_t[:])
            wseg = []  # (tile, col offset within wall)
            for name, lo, hi in (
                ("wA", 0, 2112),          # xt + first half of w1
                ("wB", 2112, 4160),       # rest of w1
                ("wC", 4160, 6208),       # w2
                ("wD", 6208, 7232),       # w3
            ):
                t = wp.tile([P128, hi - lo], fp16, tag=name)
                nc.sync.dma_start(t[:], w_t[:, lo:hi])
                wseg.append((t, lo))

            def wall_slice(lo, n):
                for t, off in wseg:
                    if off <= lo and lo + n <= off + t.shape[1]:
                        return t[:, lo - off : lo - off + n]
                raise AssertionError("bad wall slice")

            x_s = pk8[:, PK8_X : PK8_X + IN]
            id_s = pk8[:, PK8_ID : PK8_ID + BL]

            def col(j):
                c = PK8_CST + j
                return pk8[:, c : c + 1]

            # lhsT chunk slices per layer (fp16 [128, BL] each)
            vt = [[wall_slice(XT_OFF + k * BL, BL) for k in range(NKS[0])]]

            def layer(l, svec):
                """svec: [BL, 3] tile, cols = s | q | cross (cross only l>0)."""
                nk, nout = NKS[l], NOUTS[l]
                w = 2 if l == 0 else 3
                # alpha = C2*s + k_alpha
                al = scp.tile([BL, 1], fp32, tag=f"al{l}")
                nc.vector.tensor_scalar(
                    al[:], svec[:, 0:1], col(C_C2), col(C_KA0 + l), ALU.mult, ALU.add
                )
                # delta = sum(svec * [Cb, C0, 2C0]) + k_delta
                dprod = scp.tile([BL, 3], fp32, tag=f"dp{l}")
                nc.vector.tensor_tensor(
                    dprod[:, :w],
                    svec[:, :w],
                    pk8[:, PK8_CST + C_TRI0 + 3 * l : PK8_CST + C_TRI0 + 3 * l + w],
                    ALU.mult,
                )
                de = scp.tile([BL, 1], fp32, tag=f"de{l}")
                nc.vector.tensor_reduce(
                    out=de[:], in_=dprod[:, :w], axis=AX.X, op=ALU.add
                )
                de2 = scp.tile([BL, 1], fp32, tag=f"de2{l}")
                nc.vector.tensor_scalar(
                    de2[:], de[:], col(C_KD0 + l), None, ALU.add
                )
                # P = vi' @ W.T + bhat
                Pt = pp.tile([BL, nout], fp32, tag="P")
                for k in range(nk):
                    nc.tensor.matmul(
                        Pt[:],
                        vt[l][k],
                        wall_slice(W_OFF[l] + k * nout, nout),
                        start=(k == 0),
                        stop=False,
                    )
                boff = PK1_B[l]
                nc.tensor.matmul(
                    Pt[:],
                    pk1[:, PK1_ONES : PK1_ONES + BL],
                    pk1[:, boff : boff + nout],
                    start=False,
                    stop=True,
                )
                if l < 2:
                    # keep the PE HAM clock-gate open through the epilogue gap
                    for _ in range(3):
                        nc.tensor.matmul(
                            warm_p[:], junk_a[:, :BL], junk_w[:],
                            start=True, stop=True,
                        )
                # epilogue: out' = relu(P*alpha) + (C1*P + delta)   [alpha > 0]
                vja = ap_.tile([BL, nout], fp32, tag=f"vja{l}")
                nc.scalar.activation(
                    out=vja[:], in_=Pt[:], func=AF.Relu, scale=al[:, 0:1],
                    bias=col(C_ZERO),
                )
                tC = ap_.tile([BL, nout], fp32, tag=f"tC{l}")
                nc.vector.tensor_scalar(
                    tC[:], Pt[:], col(C_C1), de2[:, 0:1], ALU.mult, ALU.add
                )
                if l == 2:
                    # out = (vja + m4) + tC; the m4 add runs on gpsimd in
                    # parallel with tC on vector
                    gv = ap_.tile([BL, nout], fp32, tag="gv")
                    nc.gpsimd.tensor_tensor(
                        gv[:], vja[:], pk8[:, PK8_M4 : PK8_M4 + OUT], ALU.add
                    )
                    o = ap_.tile([BL, nout], fp32, tag=f"o{l}")
                    nc.vector.tensor_tensor(o[:], gv[:], tC[:], ALU.add)
                    return o, None
                o = ap_.tile([BL, nout], fp32, tag=f"o{l}")
                nc.vector.tensor_tensor(o[:], vja[:], tC[:], ALU.add)
                # transposes -> next layer's fp16 lhsT chunks (one copy)
                nch = nout // P128
                tp = tpp.tile([P128, nch * BL], fp32, tag="tp")
                for c in range(nch):
                    nc.tensor.transpose(
                        tp[:, c * BL : (c + 1) * BL],
                        o[:, c * P128 : (c + 1) * P128],
                        id_s,
                    )
                vtn = ap_.tile([P128, nch * BL], fp16, tag=f"vt{l + 1}")
                nc.any.tensor_copy(out=vtn[:], in_=tp[:])
                vt.append([vtn[:, k * BL : (k + 1) * BL] for k in range(nch)])
                # next-layer stats into svec columns
                sv = scp.tile([BL, 3], fp32, tag=f"sv{l + 1}")
                nc.vector.reduce_sum(out=sv[:, 0:1], in_=o[:], axis=AX.X)
                sq = scp.tile([BL, nout], fp32, tag=f"sq{l + 1}")
                nc.scalar.activation(
                    out=sq[:], in_=o[:], func=AF.Square, bias=col(C_ZERO),
                    accum_out=sv[:, 1:2],
                )
                crs = scp.tile([BL, nout], fp32, tag=f"crs{l + 1}")
                nc.vector.tensor_tensor(
                    crs[:], o[:], pk8[:, PK8_M[l + 1] : PK8_M[l + 1] + nout], ALU.mult
                )
                nc.vector.reduce_sum(out=sv[:, 2:3], in_=crs[:], axis=AX.X)
                return o, sv

            # layer-1 stats straight from fp32 x
            sv1 = scp.tile([BL, 3], fp32, tag="sv1")
            nc.vector.reduce_sum(out=sv1[:, 0:1], in_=x_s, axis=AX.X)
            sq0 = scp.tile([BL, IN], fp32, tag="sq0")
            nc.scalar.activation(
                out=sq0[:], in_=x_s, func=AF.Square, bias=col(C_ZERO),
                accum_out=sv1[:, 1:2],
            )

            o1, sv2 = layer(0, sv1)
            o2, sv3 = layer(1, sv2)
            o3, _ = layer(2, sv3)

            nc.sync.dma_start(out_t[:], o3[:])

            if DEBUG_TAPS:
                for name, ap in (("dbg_o1", o1[:]), ("dbg_o2", o2[:])):
                    t = nc.dram_tensor(
                        name, list(ap.shape), ap.dtype, kind="ExternalOutput"
                    )
                    nc.sync.dma_start(t[:], ap)

    nc.compile()
    return nc


def get_nc():
    if "nc" not in _NC_CACHE:
        _NC_CACHE["nc"] = _build_nc()
    return _NC_CACHE["nc"]


def _chunk_pt(a, dtype):
    """[R, C] -> [128, (R//128)*C]: row-chunks of 128 side by side."""
    r, c = a.shape
    nk = r // P128
    return np.ascontiguousarray(
        a.reshape(nk, P128, c).transpose(1, 0, 2).reshape(P128, nk * c), dtype=dtype
    )


def host_prep(x, fc1_w, fc1_b, fc2_w, fc2_b, fc3_w, fc3_b,
              conv1_w, conv1_b, conv2_w, conv2_b, batch_num):
    f32, f16 = np.float32, np.float16
    x = np.asarray(x, f32)
    fc1_w = np.asarray(fc1_w, f32)
    fc2_w = np.asarray(fc2_w, f32)
    fc3_w = np.asarray(fc3_w, f32)
    fc1_b = np.asarray(fc1_b, f32)
    fc2_b = np.asarray(fc2_b, f32)
    fc3_b = np.asarray(fc3_b, f32)

    bn = float(np.asarray(batch_num).item())
    scale = RATE / bn
    coef = (np.asarray(conv2_w, np.float64) @ np.asarray(conv1_w, np.float64))[0]
    bc = float(
        (np.asarray(conv2_w, np.float64) @ np.asarray(conv1_b, np.float64))[0]
        + np.asarray(conv2_b, np.float64)[0]
    )
    C0, C1, C2 = (scale * coef).astype(np.float64)
    Cb = scale * bc

    m2 = (-C1 * fc1_b.astype(np.float64)).astype(f32)
    m3 = (-C1 * fc2_b.astype(np.float64)).astype(f32)
    m4 = (-C1 * fc3_b.astype(np.float64)).astype(f32)
    bh1 = fc1_b
    bh2 = (fc2_b + m2 @ fc2_w.T).astype(f32)
    bh3 = (fc3_b + m3 @ fc3_w.T).astype(f32)

    ka = [1.0, 1.0 + C2 * float(m2.sum()), 1.0 + C2 * float(m3.sum())]
    kd = [
        0.0,
        C0 * float(m2 @ m2) + Cb * float(m2.sum()),
        C0 * float(m3 @ m3) + Cb * float(m3.sum()),
    ]
    cvec = np.zeros(NCONST, dtype=f32)
    cvec[C_C1], cvec[C_C2] = C1, C2
    cvec[C_KA0 : C_KA0 + 3] = ka
    cvec[C_KD0 : C_KD0 + 3] = kd
    for l in range(3):
        cvec[C_TRI0 + 3 * l : C_TRI0 + 3 * l + 3] = [
            Cb, C0, 0.0 if l == 0 else 2 * C0
        ]

    pk1 = np.zeros((1, PK1_LEN), f16)
    pk1[0, PK1_ONES : PK1_ONES + BL] = 1.0
    pk1[0, PK1_B[0] : PK1_B[0] + H1] = bh1.astype(f16)
    pk1[0, PK1_B[1] : PK1_B[1] + H2] = bh2.astype(f16)
    pk1[0, PK1_B[2] : PK1_B[2] + OUT] = bh3.astype(f16)

    wall_base = np.empty((P128, W_LEN), f16)
    wall_base[:, W_OFF[0] : W_OFF[0] + 4096] = _chunk_pt(fc1_w.T, f16)
    wall_base[:, W_OFF[1] : W_OFF[1] + 2048] = _chunk_pt(fc2_w.T, f16)
    wall_base[:, W_OFF[2] : W_OFF[2] + 1024] = _chunk_pt(fc3_w.T, f16)

    pk8_base = np.zeros((BL, PK8_LEN), f32)
    pk8_base[:, PK8_M[1] : PK8_M[1] + H1] = m2
    pk8_base[:, PK8_M[2] : PK8_M[2] + H2] = m3
    pk8_base[:, PK8_M4 : PK8_M4 + OUT] = m4
    pk8_base[:, PK8_CST : PK8_CST + NCONST] = cvec
    pk8_base[:, PK8_ID : PK8_ID + BL] = np.eye(BL, dtype=f32)

    in_maps = []
    for k in range(NCORES):
        xk = np.ascontiguousarray(x[k * BL : (k + 1) * BL], dtype=f32)
        pk8 = pk8_base.copy()
        pk8[:, PK8_X : PK8_X + IN] = xk
        wall = wall_base.copy()
        wall[:, XT_OFF : XT_OFF + XT_LEN] = _chunk_pt(xk.T.copy(), f16)
        in_maps.append({"pk1": pk1, "pk8": pk8, "wall": wall})
    return in_maps


def kernel(**inputs):
    from concourse.bass_utils import run_bass_kernel_spmd

    nc = get_nc()
    in_maps = host_prep(**inputs)
    res = run_bass_kernel_spmd(nc, in_maps, core_ids=list(range(NCORES)))
    out = np.concatenate([res.results[k]["outb"] for k in range(NCORES)], axis=0)
    return np.ascontiguousarray(out, dtype=np.float32)
